# revision 13
# baseline (speedup 1.0000x reference)
"""BinaryLinear TRN2 kernel, v2.

Computes out = inputs @ (sign(W) * scale).T + bias where
  sign(w) = +1 for w >= 0 else -1
  scale[o] = max(mean_i |W[o, i]|, 1e-6)

Problem shapes (hardcoded): inputs [8192, 4096] f32, weight [4096, 4096] f32,
bias [4096] f32 -> out [8192, 4096] f32.

Distribution: data-parallel over tokens (8 cores x 1024 tokens). Each core
gets a [1024, 4096] X slice plus full W/b and produces outT [4096, 1024]
(output transposed); the host lays the 8 blocks back into [8192, 4096].

v2 vs v1: the PE runs ONLY matmuls. All transposes go through the DMA xbar
(dma_start(transpose=True)), sign() stays on ACT, |W| row-sums and the
psum eviction (fused scale*psum + bias via tensor_scalar with per-partition
scalars) on DVE. psum holds outT tiles [128 o, 512 t]: lhsT = S^T tile
(stationary), rhs = X^T tile (moving, 512 tokens wide), so scale/bias are
per-partition quantities -- no broadcast round-trips.

Pipeline (per core):
  - X^T build: 32 chunks [128 t, 1024 k] f32 DMA -> ACT cast bf16 ->
    xbar transpose into a contiguous tmp [128, 8, 128] -> DVE copy-merge
    into the resident xt [128 kp, 32 kt, 1024 t] bf16 (xbar dst must be
    per-partition contiguous; the merge copy gives matmul a 512-wide rhs).
  - W stream, per 512-row output chunk: 4 row-tiles [128 o, 4096 k]; per
    row-tile 4 chunk DMAs -> ACT Sign (+1e-30 so sign(0)=+1) -> DVE abs
    row-sum -> xbar transpose each 2048-col half into an S^T slab
    [128 kp, 16 kt, 128 o] (contiguous dst).
  - Matmul: for ob in chunk: for k in 32: for tc in 2:
      psum[ob, tc] += slab(ob, k//16)[:, k%16, :].T @ xt[:, k, tc*512:...]
  - Evict: outT_sb = psum * scale[o] + bias[o] (per-partition scalars,
    one fused DVE tensor_scalar), DMA to outT dram.

Only X's bf16 rounding contributes error (~1.7e-3 relative): the sign
matrix is exact in bf16, accumulation is fp32 PSUM, scale/bias fp32.
"""

import os
import sys

import numpy as np

sys.path.insert(0, "/opt/trn_rl_repo")

import concourse.bass as bass
import concourse.mybir as mybir
from concourse import bacc
import concourse.tile as tile


def _ensure_ntff_hook():
    """The agent image's `antenv` lacks `axon_hooks`, which
    run_bass_kernel_spmd imports when trace=True (for HW exec timing).
    Provide the module and install the standard ctypes-based hook.
    Harmless when tracing is off (the import never fires)."""
    import types

    try:
        import antenv.axon_hooks  # noqa: F401
        return
    except ImportError:
        pass
    try:
        import antenv
    except ImportError:
        return
    mod = types.ModuleType("antenv.axon_hooks")
    state = {"hook": None}
    mod.set_axon_ntff_profile_hook = lambda h: state.update(hook=h)
    mod.get_axon_ntff_profile_hook = lambda: state["hook"]
    sys.modules["antenv.axon_hooks"] = mod
    antenv.axon_hooks = mod
    try:
        from trn_agent_boot.trn_boot import _ntff_profile_via_ctypes

        hook = _ntff_profile_via_ctypes("/opt/axon/libaxon_pjrt.so")
        if hook is not None:
            mod.set_axon_ntff_profile_hook(hook)
    except Exception:
        pass


_ensure_ntff_hook()

F32 = mybir.dt.float32
BF16 = mybir.dt.bfloat16

TOKENS = 8192
IN_FEATURES = 4096
OUT_FEATURES = 4096
N_CORES = 8


def build_nc(t_core, in_f, out_f):
    P = 128
    TC = min(t_core, 512)         # tokens per psum tile (rhs free dim; 512
                                  # is the ISA max moving-free for f32 psum)
    KH = 2048                     # k-columns per xbar call (16 k-tiles)
    WCH = 1024                    # k-columns per W/X staging DMA chunk
    assert t_core % TC == 0 and in_f % KH == 0 and out_f % P == 0
    n_tc = t_core // TC           # psum-width token chunks (1 at full size)
    t_tiles = t_core // P         # 128-token X row-tiles (8)
    k_tiles = in_f // P           # contraction tiles (32)
    n_kc = in_f // WCH            # X staging chunks along k (4)
    n_kh = in_f // KH             # xbar halves along k (2)
    kt_h = KH // P                # k-tiles per xbar half (16)
    n_ob = out_f // P             # total W row-tiles (32)

    nc = bacc.Bacc()
    x_dram = nc.dram_tensor("x", [t_core, in_f], F32, kind="ExternalInput")
    w_dram = nc.dram_tensor("w", [out_f, in_f], F32, kind="ExternalInput")
    b_dram = nc.dram_tensor("b", [out_f], F32, kind="ExternalInput")
    out_dram = nc.dram_tensor("out", [out_f, t_core], F32, kind="ExternalOutput")

    with tile.TileContext(nc) as tc:
        with (
            tc.tile_pool(name="resident", bufs=1) as resident,
            tc.tile_pool(name="xstage", bufs=2) as xstage,      # f32 [128, WCH]
            tc.tile_pool(name="xb16", bufs=2) as xb16,          # bf16 [128, WCH]
            tc.tile_pool(name="xtmp", bufs=2) as xtmp_pool,     # bf16 [128, 8, 128]
            tc.tile_pool(name="wstage", bufs=3) as wstage,      # f32 [128, WCH]
            tc.tile_pool(name="sstage", bufs=3) as sstage,      # bf16 [128, KH]
            tc.tile_pool(name="stslab", bufs=10) as stslab,     # bf16 [128, kt_h, 128]
            tc.tile_pool(name="small", bufs=6) as small,
            tc.tile_pool(name="outsb", bufs=3) as outsb,
            # PSUM: 16KB/partition total; [128, TC] f32 tiles are TC*4
            # bytes/partition each.
            tc.tile_pool(name="psum_mm", bufs=16384 // (TC * 4),
                         space="PSUM") as psum_mm,
        ):
            # resident X^T: xt[p, kt, t] = X[t, kt*128 + p] as bf16
            xt = resident.tile([P, k_tiles, t_core], BF16)
            # tiny positive bias so Sign(0 + tiny) = +1, matching the
            # reference's where(w >= 0, 1, -1)
            signbias = resident.tile([P, 1], F32)
            nc.vector.memset(signbias[:], 1e-30)
            # per-row scale / bias, partition-major: [p, g] <-> row g*128+p
            scale_cols = resident.tile([P, n_ob], F32)
            bias_cols = resident.tile([P, n_ob], F32)
            nc.gpsimd.dma_start(
                bias_cols[:], b_dram[:].rearrange("(g p) -> p g", p=P)
            )

            def build_x_chunk(kc, tb):
                """One [128 t, WCH k] chunk: load, cast, xbar-transpose,
                merge into xt."""
                xs = xstage.tile([P, WCH], F32, tag="xs")
                nc.sync.dma_start(
                    xs[:],
                    x_dram[tb * P:(tb + 1) * P, kc * WCH:(kc + 1) * WCH],
                )
                xb = xb16.tile([P, WCH], BF16, tag="xb")
                nc.scalar.activation(
                    xb[:], xs[:], mybir.ActivationFunctionType.Copy
                )
                xtm = xtmp_pool.tile([P, WCH // P, P], BF16, tag="xtmp")
                nc.scalar.dma_start(xtm[:], xb[:], transpose=True)
                nc.vector.tensor_copy(
                    xt[:, kc * (WCH // P):(kc + 1) * (WCH // P),
                       tb * P:(tb + 1) * P],
                    xtm[:],
                )

            def build_st(ob):
                """Stream one W row-tile [128 o, in_f]: sign -> S^T slabs,
                abs row-sum -> scale. Returns [slab_kh0, slab_kh1]."""
                red = small.tile([P, n_kc], F32, tag="red")
                slabs = []
                for kh in range(n_kh):
                    sn = sstage.tile([P, KH], BF16, tag="sn")
                    for ci in range(KH // WCH):
                        c = kh * (KH // WCH) + ci
                        ws = wstage.tile([P, WCH], F32, tag="ws")
                        # SWDGE queue: keeps W loads off the sync queue so
                        # they never sit behind output DMAs (HOL blocking).
                        nc.gpsimd.dma_start(
                            ws[:],
                            w_dram[ob * P:(ob + 1) * P,
                                   c * WCH:(c + 1) * WCH],
                        )
                        nc.scalar.activation(
                            sn[:, ci * WCH:(ci + 1) * WCH], ws[:],
                            mybir.ActivationFunctionType.Sign, bias=signbias[:],
                        )
                        nc.vector.tensor_reduce(
                            red[:, c:c + 1], ws[:],
                            axis=mybir.AxisListType.X, op=mybir.AluOpType.add,
                            apply_absolute_value=True,
                        )
                    slab = stslab.tile([P, kt_h, P], BF16, tag="slab")
                    nc.scalar.dma_start(slab[:], sn[:], transpose=True)
                    slabs.append(slab)
                redt = small.tile([P, 1], F32, tag="redt")
                nc.vector.tensor_reduce(
                    redt[:], red[:],
                    axis=mybir.AxisListType.X, op=mybir.AluOpType.add,
                )
                nc.vector.tensor_scalar(
                    scale_cols[:, ob:ob + 1], redt[:],
                    1.0 / in_f, 1e-6,
                    op0=mybir.AluOpType.mult, op1=mybir.AluOpType.max,
                )
                return slabs

            def mm_block(ob, slabs):
                """All matmuls + evictions for one 128-row output tile.
                k outer / token-chunk inner: consecutive matmuls share the
                stationary operand, so its load amortizes."""
                pms = [psum_mm.tile([P, TC], F32, tag="mmps",
                                    name=f"pm_{ob}_{tcn}")
                       for tcn in range(n_tc)]
                for k in range(k_tiles):
                    lhsT = slabs[k // kt_h][:, k % kt_h, :]
                    for tcn in range(n_tc):
                        nc.tensor.matmul(
                            pms[tcn][:], lhsT,
                            xt[:, k, tcn * TC:(tcn + 1) * TC],
                            start=(k == 0), stop=(k == k_tiles - 1),
                        )
                for tcn in range(n_tc):
                    ob_sb = outsb.tile([P, TC], F32, tag="ob")
                    nc.vector.tensor_scalar(
                        ob_sb[:], pms[tcn][:],
                        scale_cols[:, ob:ob + 1], bias_cols[:, ob:ob + 1],
                        op0=mybir.AluOpType.mult, op1=mybir.AluOpType.add,
                    )
                    nc.sync.dma_start(
                        out_dram[ob * P:(ob + 1) * P,
                                 tcn * TC:(tcn + 1) * TC],
                        ob_sb[:],
                    )

            # X chunk build order matches matmul consumption (kc-major).
            # Interleave the first W row-tiles with the X build so the
            # first matmuls can start while X streams in.
            x_chunks = [(kc, tb) for kc in range(n_kc)
                        for tb in range(t_tiles)]
            PREFETCH = 3          # W row-tiles built ahead of their mms
            pre = min(PREFETCH, n_ob)
            stride = max(1, len(x_chunks) // pre)
            slab_q = {}
            for i, ch in enumerate(x_chunks):
                build_x_chunk(*ch)
                if i % stride == stride - 1 and len(slab_q) < pre:
                    ob = len(slab_q)
                    slab_q[ob] = build_st(ob)

            for ob in range(n_ob):
                if ob + pre < n_ob:
                    slab_q[ob + pre] = build_st(ob + pre)
                mm_block(ob, slab_q.pop(ob))

    nc.finalize()
    return nc


_CACHE = {}


def kernel(inputs, weight, bias):
    from concourse.bass_utils import run_bass_kernel_spmd

    x = np.ascontiguousarray(np.asarray(inputs, dtype=np.float32))
    w = np.ascontiguousarray(np.asarray(weight, dtype=np.float32))
    b = np.ascontiguousarray(np.asarray(bias, dtype=np.float32))
    assert x.shape == (TOKENS, IN_FEATURES)
    assert w.shape == (OUT_FEATURES, IN_FEATURES)
    assert b.shape == (OUT_FEATURES,)

    if "nc" not in _CACHE:
        _CACHE["nc"] = build_nc(TOKENS // N_CORES, IN_FEATURES, OUT_FEATURES)
    nc = _CACHE["nc"]

    shards = np.split(x, N_CORES, axis=0)
    in_maps = [{"x": shards[c], "w": w, "b": b} for c in range(N_CORES)]
    trace = bool(os.environ.get("BASS_TRACE"))
    res = run_bass_kernel_spmd(nc, in_maps, list(range(N_CORES)), trace=trace)
    if trace:
        _CACHE["last_result"] = res
        if res.exec_time_ns is not None:
            print(f"HW exec time: {res.exec_time_ns} ns")

    out = np.empty((TOKENS, OUT_FEATURES), dtype=np.float32)
    tc = TOKENS // N_CORES
    for c in range(N_CORES):
        out[c * tc:(c + 1) * tc, :] = res.results[c]["out"].T
    return out
